# revision 3
# baseline (speedup 1.0000x reference)
"""MoE Top-K router kernel for Trainium2 (8 NeuronCores, data-parallel).

reference:
    logits  = X @ W.T                    # [T,H]@[H,E] -> [T,E]
    weights = softmax(logits, axis=-1)   # fp32
    indices = top_k(weights, 8).indices  # int32, sorted by weight desc

Sharding: tokens split 8 ways (2048 tokens/core); W replicated.
Host-side prep transposes each token-shard to contraction-major layout so the
device does zero transposes:
    xt[g, p, c*128 + t] = X_shard[g*128 + t, c*128 + p]   # [16,128,4096] f32
    wt[p, c*64 + e]     = W[e, c*128 + p]                 # [128,2048]  f32
Per 128-token tile the PE accumulates 32 chunked matmuls into PSUM
([128 tok, 64 exp]); ACT copies logits out + computes exp(x-max) with a free
row-sum (accum_out); DVE does max-reduce, reciprocal, scale, and the top-8
(InstMax/InstMaxIndex match jax.lax.top_k tie semantics: values descending,
ties by ascending index).
"""

import numpy as np

import concourse.bass as bass
import concourse.bacc as bacc
import concourse.mybir as mybir
from concourse.tile import TileContext
from concourse.bass_utils import run_bass_kernel_spmd

T, H, E, TOPK = 16384, 4096, 64, 8
NCORES = 8
TC = T // NCORES          # 2048 tokens per core
PT = 128                  # tokens per tile (partition dim)
NG = TC // PT             # 16 tiles per core
NCH = H // 128            # 32 contraction chunks

F32 = mybir.dt.float32
U32 = mybir.dt.uint32


def build(x_bufs: int = 4, psum_bufs: int = 4):
    nc = bacc.Bacc()
    xt = nc.dram_tensor("xt", [NG, 128, H], F32, kind="ExternalInput")
    wt = nc.dram_tensor("wt", [128, NCH * E], F32, kind="ExternalInput")
    logits = nc.dram_tensor("logits", [TC, E], F32, kind="ExternalOutput")
    weights = nc.dram_tensor("weights", [TC, E], F32, kind="ExternalOutput")
    indices = nc.dram_tensor("indices", [TC, TOPK], U32, kind="ExternalOutput")

    with TileContext(nc) as tc:
        with (
            tc.tile_pool(name="xp", bufs=x_bufs) as xp,
            tc.tile_pool(name="wp", bufs=1) as wp,
            tc.tile_pool(name="pp", bufs=psum_bufs, space="PSUM") as pp,
            tc.tile_pool(name="res", bufs=1) as res,
            tc.tile_pool(name="tmp", bufs=3) as tmp,
            tc.tile_pool(name="st", bufs=4) as st,
        ):
            wt_sb = wp.tile([128, NCH * E], F32)
            nc.sync.dma_start(out=wt_sb[:], in_=wt[:, :])

            lg_all = res.tile([128, NG * E], F32, tag="lg")
            wg_all = res.tile([128, NG * E], F32, tag="wg")
            idx_all = res.tile([128, NG * TOPK], U32, tag="idx")

            for g in range(NG):
                xg = xp.tile([128, H], F32, tag="xg")
                nc.sync.dma_start(out=xg[:], in_=xt[g])

                ps = pp.tile([128, E], F32, tag="ps")
                for c in range(NCH):
                    nc.tensor.matmul(
                        ps[:],
                        lhsT=xg[:, c * 128:(c + 1) * 128],
                        rhs=wt_sb[:, c * E:(c + 1) * E],
                        start=(c == 0),
                        stop=(c == NCH - 1),
                    )

                lg = lg_all[:, g * E:(g + 1) * E]
                nc.scalar.copy(out=lg, in_=ps[:])

                negm = st.tile([128, 1], F32, tag="negm")
                nc.vector.tensor_reduce(
                    out=negm[:], in_=lg, axis=mybir.AxisListType.X,
                    op=mybir.AluOpType.max, negate=True,
                )

                ex = tmp.tile([128, E], F32, tag="ex")
                s = st.tile([128, 1], F32, tag="s")
                nc.scalar.activation(
                    out=ex[:], in_=lg, func=mybir.ActivationFunctionType.Exp,
                    bias=negm[:], scale=1.0, accum_out=s[:],
                )

                r = st.tile([128, 1], F32, tag="r")
                nc.vector.reciprocal(r[:], s[:])

                wg = wg_all[:, g * E:(g + 1) * E]
                nc.vector.tensor_scalar_mul(wg, ex[:], r[:])

                top8 = st.tile([128, TOPK], F32, tag="top8")
                nc.vector.max(out=top8[:], in_=wg)
                idx = idx_all[:, g * TOPK:(g + 1) * TOPK]
                nc.vector.max_index(out=idx, in_max=top8[:], in_values=wg)

            # Batched writeback: dram viewed [p, g, e] to match SBUF layout.
            nc.sync.dma_start(
                out=logits.rearrange("(g p) e -> p g e", p=128), in_=lg_all[:]
            )
            nc.sync.dma_start(
                out=weights.rearrange("(g p) e -> p g e", p=128), in_=wg_all[:]
            )
            nc.sync.dma_start(
                out=indices.rearrange("(g p) k -> p g k", p=128), in_=idx_all[:]
            )

    nc.finalize()
    return nc


_NC_CACHE = None
LAST_EXEC_NS = None


def _get_nc():
    global _NC_CACHE
    if _NC_CACHE is None:
        _NC_CACHE = build()
    return _NC_CACHE


def _prep_core_inputs(hidden_states: np.ndarray, W: np.ndarray):
    wt = np.ascontiguousarray(
        W.reshape(E, NCH, 128).transpose(2, 1, 0).reshape(128, NCH * E)
    )
    in_maps = []
    for core in range(NCORES):
        xs = hidden_states[core * TC:(core + 1) * TC]
        xtc = np.ascontiguousarray(
            xs.reshape(NG, PT, NCH, 128).transpose(0, 3, 2, 1)
        ).reshape(NG, 128, H)
        in_maps.append({"xt": xtc, "wt": wt})
    return in_maps


def kernel(hidden_states: np.ndarray, W: np.ndarray):
    hidden_states = np.ascontiguousarray(hidden_states, dtype=np.float32)
    W = np.ascontiguousarray(W, dtype=np.float32)
    assert hidden_states.shape == (T, H) and W.shape == (E, H)

    nc = _get_nc()
    in_maps = _prep_core_inputs(hidden_states, W)
    res = run_bass_kernel_spmd(nc, in_maps, core_ids=list(range(NCORES)))
    global LAST_EXEC_NS
    if res.exec_time_ns is not None:
        LAST_EXEC_NS = res.exec_time_ns

    logits = np.concatenate([res.results[i]["logits"] for i in range(NCORES)], axis=0)
    weights = np.concatenate([res.results[i]["weights"] for i in range(NCORES)], axis=0)
    indices = np.concatenate(
        [res.results[i]["indices"] for i in range(NCORES)], axis=0
    ).astype(np.int32)
    return logits, weights, indices


# revision 4
# speedup vs baseline: 1.8559x; 1.8559x over previous
"""MoE Top-K router kernel for Trainium2 (8 NeuronCores, data-parallel).

reference:
    logits  = X @ W.T                    # [T,H]@[H,E] -> [T,E] fp32
    weights = softmax(logits, axis=-1)   # fp32
    indices = top_k(weights, 8).indices  # int32, sorted by weight desc

Sharding: tokens split 8 ways (2048 tokens/core); W replicated.

Matmul runs in fp16 hi/lo split form for full fp32-grade accuracy at fp16 PE
throughput (fp32 matmul is 4 cycles/row and its 4-byte weight loads don't get
FWL; fp16 gets 1 cycle/row + fast weight load):
    X = Xhi + Xlo (fp16 pair), W' = W*64 = Whi + Wlo (fp16 pair; the *64
    power-of-two prescale keeps Wlo out of fp16 subnormals and is undone
    exactly by scale=1/64 in the ACT copy/exp).
    X @ W'.T ~= Xhi@Whi + Xhi@Wlo + Xlo@Whi   (Xlo@Wlo ~ 2^-22, dropped)
Measured absmax error vs fp64 on the real data: 2.1e-6 (fp32 direct: 2.4e-6).

Host-side prep transposes each token-shard to contraction-major layout so the
device does zero transposes:
    xhl[g, p, c*128 + t]        = Xhi_shard[g*128 + t, c*128 + p]
    xhl[g, p, H + c*128 + t]    = Xlo_shard[g*128 + t, c*128 + p]
    whl[p, c*128 + e]           = Whi[e, c*128 + p]   (e < 64)
    whl[p, c*128 + 64 + e]      = Wlo[e, c*128 + p]
Per 128-token tile the PE accumulates 96 fp16 matmuls (32 chunks x 3 passes)
into one PSUM accumulator [128 tok, 64 exp]; ACT writes logits (Copy with
scale=1/64) and exp((x'-m')/64) with a free row-sum (accum_out); DVE does the
max-reduce, reciprocal, softmax scale, and top-8 (InstMax/InstMaxIndex match
jax.lax.top_k tie semantics: values descending, ties by ascending index).
"""

import numpy as np

import concourse.bass as bass
import concourse.bacc as bacc
import concourse.mybir as mybir
from concourse.tile import TileContext
from concourse.bass_utils import run_bass_kernel_spmd

T, H, E, TOPK = 16384, 4096, 64, 8
NCORES = 8
TC = T // NCORES          # 2048 tokens per core
PT = 128                  # tokens per tile (partition dim)
NG = TC // PT             # 16 tiles per core
NCH = H // 128            # 32 contraction chunks
WSCALE = 64.0             # power-of-two W prescale (exactly undone on device)

F32 = mybir.dt.float32
F16 = mybir.dt.float16
U32 = mybir.dt.uint32


def build(x_bufs: int = 4, psum_bufs: int = 4):
    nc = bacc.Bacc()
    xhl = nc.dram_tensor("xhl", [NG, 128, 2 * H], F16, kind="ExternalInput")
    whl = nc.dram_tensor("whl", [128, NCH * 128], F16, kind="ExternalInput")
    logits = nc.dram_tensor("logits", [TC, E], F32, kind="ExternalOutput")
    weights = nc.dram_tensor("weights", [TC, E], F32, kind="ExternalOutput")
    indices = nc.dram_tensor("indices", [TC, TOPK], U32, kind="ExternalOutput")

    inv = 1.0 / WSCALE

    with TileContext(nc) as tc:
        with (
            tc.tile_pool(name="xp", bufs=x_bufs) as xp,
            tc.tile_pool(name="wp", bufs=1) as wp,
            tc.tile_pool(name="pp", bufs=psum_bufs, space="PSUM") as pp,
            tc.tile_pool(name="res", bufs=1) as res,
            tc.tile_pool(name="tmp", bufs=3) as tmp,
            tc.tile_pool(name="st", bufs=4) as st,
        ):
            wt_sb = wp.tile([128, NCH * 128], F16)
            nc.sync.dma_start(out=wt_sb[:], in_=whl[:, :])

            lg_all = res.tile([128, NG * E], F32, tag="lg")
            wg_all = res.tile([128, NG * E], F32, tag="wg")
            idx_all = res.tile([128, NG * TOPK], U32, tag="idx")

            for g in range(NG):
                xg = xp.tile([128, 2 * H], F16, tag="xg")
                nc.sync.dma_start(out=xg[:], in_=xhl[g])

                ps = pp.tile([128, E], F32, tag="ps")
                n_mm = 3 * NCH
                k = 0
                for c in range(NCH):
                    hi = xg[:, c * 128:(c + 1) * 128]
                    lo = xg[:, H + c * 128:H + (c + 1) * 128]
                    whi = wt_sb[:, c * 128:c * 128 + 64]
                    wlo = wt_sb[:, c * 128 + 64:(c + 1) * 128]
                    for lhsT, rhs in ((hi, whi), (hi, wlo), (lo, whi)):
                        nc.tensor.matmul(
                            ps[:], lhsT=lhsT, rhs=rhs,
                            start=(k == 0), stop=(k == n_mm - 1),
                        )
                        k += 1

                # row max of prescaled logits (negated), then /64
                negm64 = st.tile([128, 1], F32, tag="negm64")
                nc.vector.tensor_reduce(
                    out=negm64[:], in_=ps[:], axis=mybir.AxisListType.X,
                    op=mybir.AluOpType.max, negate=True,
                )
                negm = st.tile([128, 1], F32, tag="negm")
                nc.vector.tensor_scalar_mul(negm[:], negm64[:], inv)

                lg = lg_all[:, g * E:(g + 1) * E]
                nc.scalar.activation(
                    out=lg, in_=ps[:],
                    func=mybir.ActivationFunctionType.Copy, scale=inv,
                )

                ex = tmp.tile([128, E], F32, tag="ex")
                s = st.tile([128, 1], F32, tag="s")
                nc.scalar.activation(
                    out=ex[:], in_=ps[:], func=mybir.ActivationFunctionType.Exp,
                    bias=negm[:], scale=inv, accum_out=s[:],
                )

                r = st.tile([128, 1], F32, tag="r")
                nc.vector.reciprocal(r[:], s[:])

                wg = wg_all[:, g * E:(g + 1) * E]
                nc.vector.tensor_scalar_mul(wg, ex[:], r[:])

                top8 = st.tile([128, TOPK], F32, tag="top8")
                nc.vector.max(out=top8[:], in_=wg)
                idx = idx_all[:, g * TOPK:(g + 1) * TOPK]
                nc.vector.max_index(out=idx, in_max=top8[:], in_values=wg)

            # Batched writeback: dram viewed [p, g, e] to match SBUF layout.
            nc.sync.dma_start(
                out=logits.rearrange("(g p) e -> p g e", p=128), in_=lg_all[:]
            )
            nc.sync.dma_start(
                out=weights.rearrange("(g p) e -> p g e", p=128), in_=wg_all[:]
            )
            nc.sync.dma_start(
                out=indices.rearrange("(g p) k -> p g k", p=128), in_=idx_all[:]
            )

    nc.finalize()
    return nc


_NC_CACHE = None
LAST_EXEC_NS = None


def _get_nc():
    global _NC_CACHE
    if _NC_CACHE is None:
        _NC_CACHE = build()
    return _NC_CACHE


def _prep_core_inputs(hidden_states: np.ndarray, W: np.ndarray):
    # W prescale + fp16 hi/lo split, contraction-major: whl[p, c, 0:64|64:128]
    Wp = (W * WSCALE).astype(np.float32)
    W1 = np.ascontiguousarray(Wp.reshape(E, NCH, 128).transpose(2, 1, 0))  # [128,NCH,64]
    whi = W1.astype(np.float16)
    wlo = (W1 - whi.astype(np.float32)).astype(np.float16)
    whl = np.concatenate([whi, wlo], axis=2).reshape(128, NCH * 128)

    in_maps = []
    for core in range(NCORES):
        xs = hidden_states[core * TC:(core + 1) * TC]
        xt = np.ascontiguousarray(
            xs.reshape(NG, PT, NCH, 128).transpose(0, 3, 2, 1)
        ).reshape(NG, 128, H)                      # [g, p(h), c*128+t] fp32
        xhi = xt.astype(np.float16)
        xlo = (xt - xhi.astype(np.float32)).astype(np.float16)
        xhl = np.concatenate([xhi, xlo], axis=2)   # [g, p, 2H] fp16
        in_maps.append({"xhl": xhl, "whl": whl})
    return in_maps


def kernel(hidden_states: np.ndarray, W: np.ndarray):
    hidden_states = np.ascontiguousarray(hidden_states, dtype=np.float32)
    W = np.ascontiguousarray(W, dtype=np.float32)
    assert hidden_states.shape == (T, H) and W.shape == (E, H)

    nc = _get_nc()
    in_maps = _prep_core_inputs(hidden_states, W)
    res = run_bass_kernel_spmd(nc, in_maps, core_ids=list(range(NCORES)))
    global LAST_EXEC_NS
    if res.exec_time_ns is not None:
        LAST_EXEC_NS = res.exec_time_ns

    logits = np.concatenate([res.results[i]["logits"] for i in range(NCORES)], axis=0)
    weights = np.concatenate([res.results[i]["weights"] for i in range(NCORES)], axis=0)
    indices = np.concatenate(
        [res.results[i]["indices"] for i in range(NCORES)], axis=0
    ).astype(np.int32)
    return logits, weights, indices


# revision 6
# speedup vs baseline: 2.1309x; 1.1481x over previous
"""MoE Top-K router kernel for Trainium2 (8 NeuronCores, data-parallel).

reference:
    logits  = X @ W.T                    # [T,H]@[H,E] -> [T,E] fp32
    weights = softmax(logits, axis=-1)   # fp32
    indices = top_k(weights, 8).indices  # int32, sorted by weight desc

Sharding: tokens split 8 ways (2048 tokens/core); W replicated.

Matmul runs in fp16 hi/lo split form for full fp32-grade accuracy at fp16 PE
throughput (fp32 matmul is 4 cycles/row and its 4-byte weight loads don't get
FWL; fp16 gets 1 cycle/row + fast weight load):
    X = Xhi + Xlo (fp16 pair), W' = W*64 = Whi + Wlo (fp16 pair; the *64
    power-of-two prescale keeps Wlo out of fp16 subnormals and is undone
    exactly by scale=1/64 in the ACT copy/exp).
    X @ W'.T ~= Xhi@Whi + Xhi@Wlo + Xlo@Whi   (Xlo@Wlo ~ 2^-22, dropped)
Measured absmax error vs fp64 on the real data: 2.1e-6 (fp32 direct: 2.4e-6).

Host-side prep transposes each token-shard to contraction-major layout so the
device does zero transposes:
    xhl[g, p, c*128 + t]        = Xhi_shard[g*128 + t, c*128 + p]
    xhl[g, p, H + c*128 + t]    = Xlo_shard[g*128 + t, c*128 + p]
    whl[p, c*128 + e]           = Whi[e, c*128 + p]   (e < 64)
    whl[p, c*128 + 64 + e]      = Wlo[e, c*128 + p]
Per 128-token tile the PE accumulates 96 fp16 matmuls (32 chunks x 3 passes)
into one PSUM accumulator [128 tok, 64 exp]; ACT writes logits (Copy with
scale=1/64) and exp((x'-m')/64) with a free row-sum (accum_out); DVE does the
max-reduce, reciprocal, softmax scale, and top-8 (InstMax/InstMaxIndex match
jax.lax.top_k tie semantics: values descending, ties by ascending index).
"""

import numpy as np

import concourse.bass as bass
import concourse.bacc as bacc
import concourse.mybir as mybir
from concourse.tile import TileContext
from concourse.bass_utils import run_bass_kernel_spmd

T, H, E, TOPK = 16384, 4096, 64, 8
NCORES = 8
TC = T // NCORES          # 2048 tokens per core
PT = 128                  # tokens per tile (partition dim)
NG = TC // PT             # 16 tiles per core
NCH = H // 128            # 32 contraction chunks
WSCALE = 64.0             # power-of-two W prescale (exactly undone on device)

F32 = mybir.dt.float32
F16 = mybir.dt.float16
U32 = mybir.dt.uint32


def build(x_bufs: int = 6, psum_bufs: int = 4, out_every: int = 4):
    nc = bacc.Bacc()
    xhl = nc.dram_tensor("xhl", [NG, 128, 2 * H], F16, kind="ExternalInput")
    whl = nc.dram_tensor("whl", [128, NCH * 128], F16, kind="ExternalInput")
    logits = nc.dram_tensor("logits", [TC, E], F32, kind="ExternalOutput")
    weights = nc.dram_tensor("weights", [TC, E], F32, kind="ExternalOutput")
    indices = nc.dram_tensor("indices", [TC, TOPK], U32, kind="ExternalOutput")

    inv = 1.0 / WSCALE
    lg_view = logits.rearrange("(g p) e -> p g e", p=128)
    wg_view = weights.rearrange("(g p) e -> p g e", p=128)
    ix_view = indices.rearrange("(g p) k -> p g k", p=128)

    with TileContext(nc) as tc:
        with (
            tc.tile_pool(name="xp", bufs=x_bufs) as xp,
            tc.tile_pool(name="wp", bufs=1) as wp,
            tc.tile_pool(name="pp", bufs=psum_bufs, space="PSUM") as pp,
            tc.tile_pool(name="res", bufs=1) as res,
            tc.tile_pool(name="tmp", bufs=3) as tmp,
            tc.tile_pool(name="st", bufs=4) as st,
        ):
            # weights on the ACT HWDGE ring so the SP ring starts streaming X
            # immediately
            wt_sb = wp.tile([128, NCH * 128], F16)
            nc.scalar.dma_start(out=wt_sb[:], in_=whl[:, :])

            lg_all = res.tile([128, NG * E], F32, tag="lg")
            wg_all = res.tile([128, NG * E], F32, tag="wg")
            idx_all = res.tile([128, NG * TOPK], U32, tag="idx")

            for g in range(NG):
                xg = xp.tile([128, 2 * H], F16, tag="xg")
                nc.sync.dma_start(out=xg[:], in_=xhl[g])

                # ps cols 0:64 accumulate Xhi@Whi + Xlo@Whi, cols 64:128
                # accumulate Xhi@Wlo; one hi weight-load covers both hi passes
                # (rhs = [Whi|Wlo] concat, N=128).
                ps = pp.tile([128, 2 * E], F32, tag="ps")
                for c in range(NCH):
                    hi = xg[:, c * 128:(c + 1) * 128]
                    lo = xg[:, H + c * 128:H + (c + 1) * 128]
                    whilo = wt_sb[:, c * 128:(c + 1) * 128]
                    whi = wt_sb[:, c * 128:c * 128 + 64]
                    nc.tensor.matmul(
                        ps[:], lhsT=hi, rhs=whilo,
                        start=(c == 0), stop=False, skip_group_check=True,
                    )
                    nc.tensor.matmul(
                        ps[:, 0:E], lhsT=lo, rhs=whi,
                        start=False, stop=(c == NCH - 1), skip_group_check=True,
                    )

                # fold the two column halves: x' = 64 * logits
                # (tensor_tensor may read only one PSUM operand -> bounce one
                # half through SBUF on ACT)
                half = tmp.tile([128, E], F32, tag="half")
                nc.scalar.activation(
                    out=half[:], in_=ps[:, E:2 * E],
                    func=mybir.ActivationFunctionType.Copy,
                )
                lgp = tmp.tile([128, E], F32, tag="lgp")
                nc.vector.tensor_add(lgp[:], ps[:, 0:E], half[:])

                # row max of prescaled logits (negated), then /64
                negm64 = st.tile([128, 1], F32, tag="negm64")
                nc.vector.tensor_reduce(
                    out=negm64[:], in_=lgp[:], axis=mybir.AxisListType.X,
                    op=mybir.AluOpType.max, negate=True,
                )
                negm = st.tile([128, 1], F32, tag="negm")
                nc.vector.tensor_scalar_mul(negm[:], negm64[:], inv)

                lg = lg_all[:, g * E:(g + 1) * E]
                nc.scalar.activation(
                    out=lg, in_=lgp[:],
                    func=mybir.ActivationFunctionType.Copy, scale=inv,
                )

                ex = tmp.tile([128, E], F32, tag="ex")
                s = st.tile([128, 1], F32, tag="s")
                nc.scalar.activation(
                    out=ex[:], in_=lgp[:], func=mybir.ActivationFunctionType.Exp,
                    bias=negm[:], scale=inv, accum_out=s[:],
                )

                r = st.tile([128, 1], F32, tag="r")
                nc.vector.reciprocal(r[:], s[:])

                wg = wg_all[:, g * E:(g + 1) * E]
                nc.vector.tensor_scalar_mul(wg, ex[:], r[:])

                top8 = st.tile([128, TOPK], F32, tag="top8")
                nc.vector.max(out=top8[:], in_=wg)
                idx = idx_all[:, g * TOPK:(g + 1) * TOPK]
                nc.vector.max_index(out=idx, in_max=top8[:], in_values=wg)

                # incremental writeback on the ACT ring, overlapped with the
                # input stream on the SP ring
                if (g + 1) % out_every == 0:
                    g0 = g + 1 - out_every
                    gs = slice(g0, g + 1)
                    fs = slice(g0 * E, (g + 1) * E)
                    ks = slice(g0 * TOPK, (g + 1) * TOPK)
                    nc.scalar.dma_start(out=lg_view[:, gs, :], in_=lg_all[:, fs])
                    nc.scalar.dma_start(out=wg_view[:, gs, :], in_=wg_all[:, fs])
                    nc.scalar.dma_start(out=ix_view[:, gs, :], in_=idx_all[:, ks])

    nc.finalize()
    return nc


_NC_CACHE = None
LAST_EXEC_NS = None


def _get_nc():
    global _NC_CACHE
    if _NC_CACHE is None:
        _NC_CACHE = build()
    return _NC_CACHE


def _prep_core_inputs(hidden_states: np.ndarray, W: np.ndarray):
    # W prescale + fp16 hi/lo split, contraction-major: whl[p, c, 0:64|64:128]
    Wp = (W * WSCALE).astype(np.float32)
    W1 = np.ascontiguousarray(Wp.reshape(E, NCH, 128).transpose(2, 1, 0))  # [128,NCH,64]
    whi = W1.astype(np.float16)
    wlo = (W1 - whi.astype(np.float32)).astype(np.float16)
    whl = np.concatenate([whi, wlo], axis=2).reshape(128, NCH * 128)

    in_maps = []
    for core in range(NCORES):
        xs = hidden_states[core * TC:(core + 1) * TC]
        xt = np.ascontiguousarray(
            xs.reshape(NG, PT, NCH, 128).transpose(0, 3, 2, 1)
        ).reshape(NG, 128, H)                      # [g, p(h), c*128+t] fp32
        xhi = xt.astype(np.float16)
        xlo = (xt - xhi.astype(np.float32)).astype(np.float16)
        xhl = np.concatenate([xhi, xlo], axis=2)   # [g, p, 2H] fp16
        in_maps.append({"xhl": xhl, "whl": whl})
    return in_maps


def kernel(hidden_states: np.ndarray, W: np.ndarray):
    hidden_states = np.ascontiguousarray(hidden_states, dtype=np.float32)
    W = np.ascontiguousarray(W, dtype=np.float32)
    assert hidden_states.shape == (T, H) and W.shape == (E, H)

    nc = _get_nc()
    in_maps = _prep_core_inputs(hidden_states, W)
    res = run_bass_kernel_spmd(nc, in_maps, core_ids=list(range(NCORES)))
    global LAST_EXEC_NS
    if res.exec_time_ns is not None:
        LAST_EXEC_NS = res.exec_time_ns

    logits = np.concatenate([res.results[i]["logits"] for i in range(NCORES)], axis=0)
    weights = np.concatenate([res.results[i]["weights"] for i in range(NCORES)], axis=0)
    indices = np.concatenate(
        [res.results[i]["indices"] for i in range(NCORES)], axis=0
    ).astype(np.int32)
    return logits, weights, indices
